# revision 81
# baseline (speedup 1.0000x reference)
"""Trainium2 Bass kernel for a binarized BasicBlock (2x bconv3x3 + BN +
residual hardtanh + channel shuffle), data-parallel over batch on 8 cores.

Self-contained: hardcodes shapes from the problem spec.
  x: (32, 256, 56, 56) f32 -> out: (32, 256, 56, 56) f32

v2 design notes:
- conv2 of image n trails conv1 of image n by 2 tile-pairs, so the PE
  matmul stream never gaps (HAM stays warm at 2.4 GHz).
- epilogue is fused hard: move1_even folds into conv1's ACT bias for hi
  partitions; the fo1 output clip uses shifted per-partition bounds;
  conv2's residual/binarize source XR = [clip(t13_lo) ; idle_lo] is
  assembled in place so u2 and the conv2 residual add are single
  128-partition ops; move0 folds into the u2 threshold and conv2 bias.
- DMA: sync queue carries only loads (incl. one packed f8 weight blob),
  scalar queue carries all stores, emitted as soon as data is ready.
"""

import numpy as np
import ml_dtypes

import concourse.bass as bass
import concourse.tile as tile
from concourse import bacc, mybir
from concourse import bass_utils

EPS = 1e-5
P = 128
H = W = 56
HW = H * W
WP = 64          # padded row width (DoubleRow needs 16B-aligned row pitch)
RP = 59          # padded rows allocated (58 used + 1 spare for tail reads)
IMGS_PER_CORE = 4
NCORES = 8
TF = 8 * WP      # matmul free size per tile = 512 (= one PSUM bank)

F32 = mybir.dt.float32
BF16 = mybir.dt.bfloat16
FP16 = mybir.dt.float16
F8 = mybir.dt.float8e4
ALU = mybir.AluOpType
ACTF = mybir.ActivationFunctionType

_CACHE = {}


def _flat(ap3):
    return ap3.rearrange("p r c -> p (r c)")


def _build():
    nc = bacc.Bacc("TRN2", target_bir_lowering=False, debug=False)

    x_h = nc.dram_tensor("xs", [IMGS_PER_CORE, 2 * P, H, W], F32, kind="ExternalInput")
    wcb_h = nc.dram_tensor("wcb", [P, 2 * 9 * P], F8, kind="ExternalInput")
    cst_h = nc.dram_tensor("cst", [P, 16], F32, kind="ExternalInput")
    dg2_h = nc.dram_tensor("dg2", [P, P], FP16, kind="ExternalInput")
    out_h = nc.dram_tensor("out", [IMGS_PER_CORE, 2 * P, H, W], F32, kind="ExternalOutput")

    x_ap = x_h.ap()

    def st_ch4(n, base_ch, e0=0, e1=HW):
        # channels base_ch, base_ch+4, ..., elements e0..e1 of each;
        # keep the DRAM AP 2D — deeper APs fall off the DGE fast path and
        # cost 5-18us of engine time per trigger.
        return bass.AP(
            tensor=out_h,
            offset=(n * 2 * P + base_ch) * HW + e0,
            ap=[[4 * HW, 64], [1, e1 - e0]],
        )

    with tile.TileContext(nc) as tc:
        XA = [nc.alloc_sbuf_tensor(f"XA{i}", [P, H, W], F32).ap() for i in range(2)]
        T13 = [nc.alloc_sbuf_tensor(f"T13{i}", [P, H, W], F32).ap() for i in range(2)]
        XR = [nc.alloc_sbuf_tensor(f"XR{i}", [P, H, W], F32).ap() for i in range(3)]
        # conv2 residual in bf16 for the PE prefill; +64 spare elements so the
        # last tile's overlapped-row rhs read stays in bounds
        XRb = [nc.alloc_sbuf_tensor(f"XRb{i}", [P, HW + 64], FP16).ap() for i in range(2)]
        FXI = [nc.alloc_sbuf_tensor(f"FXI{i}", [P, H, W], F32).ap() for i in range(2)]
        OT2 = [nc.alloc_sbuf_tensor(f"OT2{i}", [P, H, W], F32).ap() for i in range(2)]
        B1 = [nc.alloc_sbuf_tensor(f"B1{i}", [P, RP, WP], F8).ap() for i in range(2)]
        B2 = [nc.alloc_sbuf_tensor(f"B2{i}", [P, RP, WP], F8).ap() for i in range(2)]
        WCB = nc.alloc_sbuf_tensor("WCB", [P, 2 * 9 * P], F8).ap()
        CST = nc.alloc_sbuf_tensor("CST", [P, 16], F32).ap()
        DG2 = nc.alloc_sbuf_tensor("DG2", [P, P], FP16).ap()

        WS1 = WCB[:, 0:9 * P]
        WS2 = WCB[:, 9 * P:18 * P]
        s1 = CST[:, 0:1]
        b1 = CST[:, 1:2]          # includes +move1_even on hi partitions
        s2 = CST[:, 2:3]
        b2 = CST[:, 3:4]          # includes +move0 on hi partitions
        fo_hi = CST[64:128, 4:5]  # 1 + move1_even
        fo_lo = CST[64:128, 5:6]  # -1 + move1_even
        th2 = CST[:, 6:7]         # u2 threshold: 0 (lo) / -move0 (hi)
        cxh = CST[0:64, 7:8]      # move0_hi + move1_odd

        # loads on the sync HW queue: first image's active half in row
        # chunks so conv1(0) starts early; weights+consts go on the scalar
        # queue concurrently so they never block the x stream.
        for r0, r1 in ((0, 16), (16, 32), (32, 48), (48, 56)):
            nc.sync.dma_start(out=XA[0][:, r0:r1], in_=x_ap[0, 0:P, r0:r1])
        nc.scalar.dma_start(out=WCB, in_=wcb_h.ap())
        nc.scalar.dma_start(out=CST, in_=cst_h.ap())
        nc.scalar.dma_start(out=DG2, in_=dg2_h.ap())
        nc.sync.dma_start(out=XR[0][64:128], in_=x_ap[0, P:P + 64],
                          max_dma_last_dim=1568)
        nc.sync.dma_start(out=FXI[0][0:64], in_=x_ap[0, P + 64:2 * P],
                          max_dma_last_dim=1568)
        nc.sync.dma_start(out=XA[1], in_=x_ap[1, 0:P], max_dma_last_dim=1568)
        nc.sync.dma_start(out=XR[1][64:128], in_=x_ap[1, P:P + 64],
                          max_dma_last_dim=1568)
        nc.sync.dma_start(out=FXI[1][0:64], in_=x_ap[1, P + 64:2 * P],
                          max_dma_last_dim=1568)

        # u-domain pads: 0.5 stands for binarized zero-padding.
        for _b in (*B1, *B2):
            _f = _flat(_b)
            nc.gpsimd.memset(_f[:, 0:WP], 0.5)
            nc.gpsimd.memset(_f[:, 57 * WP:RP * WP], 0.5)
            nc.gpsimd.memset(_b[:, 1:57, 0:1], 0.5)
            nc.gpsimd.memset(_b[:, 1:57, 57:64], 0.5)
        for _x in XRb:
            nc.gpsimd.memset(_x[:, HW:HW + 64], 0.0)

        with (
            tc.tile_pool(name="psA", bufs=2, space="PSUM") as psA,
            tc.tile_pool(name="psB", bufs=2, space="PSUM") as psB,
            tc.tile_pool(name="t23p", bufs=3) as t23p,
        ):
            def emit_conv_mms(ps, ws, bf, tp, npair, resid=None, tiles=None):
                """3 vertical DoubleRow tap-pairs + 3 single taps per tile,
                weight-outer across the tile pair so identical LDWEIGHTS
                are back to back. If resid is given (conv2), the group is
                opened by a diag(1/s2) @ resid bf16 matmul that deposits the
                pre-scaled residual into PSUM."""
                DR = mybir.MatmulPerfMode.DoubleRow
                tl = list(range(npair)) if tiles is None else tiles
                for g in range(3):  # pairs: taps (0,g)+(1,g), delta = WP
                    lhsT = bass.AP(tensor=ws.tensor, offset=ws.offset + 256 * g,
                                   ap=[list(ws.ap[0]), [P, 2], [1, P]])
                    for j in tl:
                        base = (8 * (2 * tp + j)) * WP + g
                        rhs = bass.AP(tensor=bf.tensor, offset=bf.offset + base,
                                      ap=[list(bf.ap[0]), [WP, 2], [1, TF]])
                        nc.tensor.matmul(
                            ps[:, j, :], lhsT=lhsT, rhs=rhs,
                            start=(g == 0), stop=False, perf_mode=DR)
                for g in range(3):  # single taps (2,g): 464-col APs skip the
                    # 6 junk pad columns per row (the DR start matmul already
                    # set has_written on them, so they just keep junk)
                    lhsT = ws[:, 768 + P * g:768 + P * (g + 1)]
                    for j in tl:
                        off = (8 * (2 * tp + j) + 2) * WP + g
                        rhs = bass.AP(tensor=bf.tensor, offset=bf.offset + off,
                                      ap=[list(bf.ap[0]), [WP, 8], [1, 58]])
                        dst = bass.AP(tensor=ps.tensor,
                                      offset=ps.offset + j * 512,
                                      ap=[list(ps.ap[0]), [WP, 8], [1, 58]])
                        nc.tensor.matmul(
                            dst, lhsT=lhsT, rhs=rhs,
                            start=False, stop=(g == 2 and resid is None))
                # residual deposit LAST so the XRb dependency gates only the
                # end of the group, not its start
                if resid is not None:
                    for j in tl:
                        r0 = 8 * (2 * tp + j)
                        rhs = bass.AP(tensor=resid.tensor,
                                      offset=resid.offset + r0 * W,
                                      ap=[list(resid.ap[0]), [W, 8], [1, 58]])
                        dst = bass.AP(tensor=ps.tensor,
                                      offset=ps.offset + j * 512,
                                      ap=[list(ps.ap[0]), [WP, 8], [1, 58]])
                        nc.tensor.matmul(dst, lhsT=DG2, rhs=rhs,
                                         start=False, stop=(j == tl[-1]))

            def u1(n, r0, r1):
                nc.vector.tensor_scalar(
                    out=B1[n % 2][:, 1 + r0:1 + r1, 1:57], in0=XA[n % 2][:, r0:r1],
                    scalar1=0.0, scalar2=None, op0=ALU.is_ge)

            def slotA(n, tp):
                """conv1 pair tp of image n: matmuls + fused epilogue.
                Gating chain for conv2 is only ACT1 -> TT1 -> u2a (binarize
                reads the UNCLIPPED t13 lo: sign(clip(x)) == sign(x));
                the clip into XR is needed much later (conv2 residual)."""
                s = n % 2
                s3 = n % 3
                npair = 2 if tp < 3 else 1
                nr = 8 * npair
                r0 = 16 * tp
                e0, e1 = r0 * W, (r0 + nr) * W
                ps = psA.tile([P, 2, 512], F32)
                emit_conv_mms(ps, WS1, _flat(B1[s]), tp, npair)
                ps3 = _flat(ps).rearrange("p (r c) -> p r c", c=WP)[:, 0:nr, 0:W]
                t13 = _flat(T13[s])
                xaf = _flat(XA[s])
                nc.scalar.activation(t13[:, e0:e1], ps3, ACTF.Identity,
                                     bias=b1, scale=s1)
                nc.vector.tensor_tensor(
                    out=t13[:, e0:e1], in0=t13[:, e0:e1], in1=xaf[:, e0:e1],
                    op=ALU.add)
                # u2a chunk: B2 lo rows from unclipped t13 lo
                nc.vector.tensor_scalar(
                    out=B2[s][0:64, 1 + r0:1 + r0 + nr, 1:57],
                    in0=T13[s][0:64, r0:r0 + nr, :],
                    scalar1=0.0, scalar2=None, op0=ALU.is_ge)
                # lo: clip -> bf16 XRb (conv2 PE-prefill residual)
                nc.gpsimd.tensor_scalar(
                    out=XRb[s][0:64, e0:e1], in0=t13[0:64, e0:e1],
                    scalar1=1.0, scalar2=-1.0, op0=ALU.min, op1=ALU.max)
                # hi: clip with shifted bounds == clip(t13hi - m) + m where
                # m = move1_even was pre-added via the ACT bias
                nc.gpsimd.tensor_scalar(
                    out=_flat(FXI[s])[64:128, e0:e1], in0=t13[64:128, e0:e1],
                    scalar1=fo_hi, scalar2=fo_lo, op0=ALU.min, op1=ALU.max)

            def u2b(n):
                # whole-image idle-half binarize, off the critical chain
                nc.vector.tensor_scalar(
                    out=B2[n % 2][64:128, 1:57, 1:57],
                    in0=XR[n % 3][64:128, :, :],
                    scalar1=th2[64:128], scalar2=None, op0=ALU.is_ge)
                # idle half of the conv2 residual, as bf16 for the prefill
                nc.vector.tensor_scalar(
                    out=XRb[n % 2][64:128, 0:HW], in0=_flat(XR[n % 3])[64:128],
                    scalar1=0.0, scalar2=None, op0=ALU.add)

            def slotB_epi(n, tp, ps):
                s = n % 2
                npair = 2 if tp < 3 else 1
                nr = 8 * npair
                r0 = 16 * tp
                e0, e1 = r0 * W, (r0 + nr) * W
                ps3 = _flat(ps).rearrange("p (r c) -> p r c", c=WP)[:, 0:nr, 0:W]
                t23f = t23p.tile([P, 16 * W], F32, tag="t23", name="t23")
                t23 = t23f[:, 0:nr * W]
                nc.scalar.activation(t23, ps3, ACTF.Identity, bias=b2, scale=s2)
                nc.vector.tensor_scalar(
                    out=_flat(OT2[s])[:, e0:e1], in0=t23,
                    scalar1=1.0, scalar2=-1.0, op0=ALU.min, op1=ALU.max)

            def slotB(n, tp):
                """conv2 pair tp of image n."""
                s = n % 2
                npair = 2 if tp < 3 else 1
                ps = psB.tile([P, 2, 512], F32)
                emit_conv_mms(ps, WS2, _flat(B2[s]), tp, npair, resid=XRb[s])
                slotB_epi(n, tp, ps)

            def slotB_tile(n, tp, t, ps=None):
                """single tile t of conv2 pair tp (image-0 ramp: tile 0 is
                gated by already-ready B2 rows while tile 1's last row is
                still in flight)."""
                s = n % 2
                if ps is None:
                    ps = psB.tile([P, 2, 512], F32)
                emit_conv_mms(ps, WS2, _flat(B2[s]), tp, 2, resid=XRb[s],
                              tiles=[t])
                return ps

            def xih_act(n, half):
                # idle-hi passthrough bias, in place, in halves
                f = _flat(FXI[n % 2])
                e0, e1 = half * 28 * W, (half + 1) * 28 * W
                nc.scalar.activation(f[0:64, e0:e1], f[0:64, e0:e1],
                                     ACTF.Identity, bias=cxh, scale=1.0)

            def load_xa(n):
                if n >= IMGS_PER_CORE:
                    return
                nc.sync.dma_start(out=XA[n % 2], in_=x_ap[n, 0:P],
                                  max_dma_last_dim=1568)

            def load_idl(n):
                if n >= IMGS_PER_CORE:
                    return
                nc.sync.dma_start(out=XR[n % 3][64:128], in_=x_ap[n, P:P + 64],
                                  max_dma_last_dim=1568)

            def load_xih(n):
                if n >= IMGS_PER_CORE:
                    return
                nc.sync.dma_start(out=FXI[n % 2][0:64], in_=x_ap[n, P + 64:2 * P],
                                  max_dma_last_dim=1568)

            def st_fo1(n):
                # scalar queue: data is ready well before the trigger point,
                # so it never head-of-line blocks the ACT stream
                nc.scalar.dma_start(out=st_ch4(n, 1), in_=_flat(FXI[n % 2])[64:128])

            def st_xih(n):
                nc.scalar.dma_start(out=st_ch4(n, 3), in_=_flat(FXI[n % 2])[0:64])

            def store_ot2(n, r0, r1):
                # last image: split the two halves across both HW queues so
                # tail trigger issue isn't serialized on one engine
                e0, e1 = r0 * W, r1 * W
                f = _flat(OT2[n % 2])
                nc.sync.dma_start(out=st_ch4(n, 0, e0, e1), in_=f[0:64, e0:e1])
                nc.sync.dma_start(out=st_ch4(n, 2, e0, e1), in_=f[64:128, e0:e1])

            # --- prologue: image 0 u1 per load chunk ---
            scope = nc.named_scope
            with scope("pro"):
                for r0, r1 in ((0, 16), (16, 32), (32, 48), (48, 56)):
                    u1(0, r0, r1)

            # slot schedule, conv2 lags conv1 by 3 pairs:
            # A00 A01 A02 A03 B00 A10 B01 A11 B02 A12 B03 A13 B10 A20 ...
            with scope("a00"):
                slotA(0, 0)
                u2b(0)
            with scope("a01"):
                slotA(0, 1)
                xih_act(0, 0)
            with scope("a02"):
                slotA(0, 2)
                u1(1, 0, 32)
                xih_act(0, 1)
                st_xih(0)
            with scope("a03"):
                slotA(0, 3)
                u1(1, 32, 56)
            def a_zero(nx):
                with scope(f"a{nx}0"):
                    u2b(nx)            # first: zero deps, gates B(nx,0)
                    slotA(nx, 0)
                    load_xa(nx + 1)
                    load_idl(nx + 1)   # XR 3-deep: no WAR conflict
                    load_xih(nx + 1)   # after st_xih(nx-1) trigger

            # A(n+1, p) is emitted BEFORE B(n, p): conv2 trails conv1 by a
            # full image of pairs, so every gating chain has ~8 slots of
            # slack while the PE still alternates A/B matmul pairs.
            # image-0 ramp: conv2 pairs 0 and 1 are emitted tile-by-tile so
            # tile 0 (gated on already-finished B2 rows) fills what would be
            # a PE gap while pair p+1's epilogue chain finishes
            a_zero(1)
            with scope("b00s"):
                ps00 = slotB_tile(0, 0, 0)
            with scope("a11"):
                slotA(1, 1)
                xih_act(1, 0)
            with scope("b00f"):
                slotB_tile(0, 0, 1, ps00)
                slotB_epi(0, 0, ps00)
            with scope("b01s"):
                ps01 = slotB_tile(0, 1, 0)
            with scope("a12"):
                u1(2, 0, 32)
                u1(2, 32, 56)
                slotA(1, 2)
                xih_act(1, 1)
                st_xih(1)
            with scope("b01f"):
                slotB_tile(0, 1, 1, ps01)
                slotB_epi(0, 1, ps01)
                st_fo1(0)
            for n in range(IMGS_PER_CORE):
                last = n == IMGS_PER_CORE - 1
                nx = n + 1
                if n > 0:
                    if not last:
                        a_zero(nx)
                    with scope(f"b{n}0"):
                        slotB(n, 0)
                    if not last:
                        with scope(f"a{nx}1"):
                            slotA(nx, 1)
                            xih_act(nx, 0)
                    with scope(f"b{n}1"):
                        slotB(n, 1)
                        st_fo1(n)
                        if last:
                            store_ot2(n, 0, 16)
                if not last and n > 0:
                    with scope(f"a{nx}2"):
                        if nx < IMGS_PER_CORE - 1:
                            u1(nx + 1, 0, 32)
                            u1(nx + 1, 32, 56)
                        slotA(nx, 2)
                        xih_act(nx, 1)
                        st_xih(nx)
                with scope(f"b{n}2"):
                    slotB(n, 2)
                    if last:
                        store_ot2(n, 16, 32)
                    else:
                        store_ot2(n, 0, 32)
                if not last:
                    with scope(f"a{nx}3"):
                        slotA(nx, 3)
                with scope(f"b{n}3"):
                    slotB(n, 3)
                    if last:
                        e0, e1 = 32 * W, 48 * W
                        f = _flat(OT2[n % 2])
                        nc.sync.dma_start(out=st_ch4(n, 0, e0, e1),
                                          in_=f[0:64, e0:e1])
                        nc.scalar.dma_start(out=st_ch4(n, 2, e0, e1),
                                            in_=f[64:128, e0:e1])
                    else:
                        store_ot2(n, 32, 56)
            with scope("tail"):
                e0, e1 = 48 * W, HW
                f = _flat(OT2[3 % 2])
                nc.sync.dma_start(out=st_ch4(3, 0, e0, e1), in_=f[0:64, e0:e1])
                nc.scalar.dma_start(out=st_ch4(3, 2, e0, e1), in_=f[64:128, e0:e1])

    nc.compile()
    return nc


def _host_prep(w1, w2, bn1_gamma, bn1_beta, bn1_mean, bn1_var,
               bn2_gamma, bn2_beta, bn2_mean, bn2_var, move0_bias, move1_bias):
    f8 = np.float64
    bw1 = np.where(w1 >= 0, 1.0, -1.0).astype(f8)   # [co, ci, 3, 3]
    bw2 = np.where(w2 >= 0, 1.0, -1.0).astype(f8)

    def wlayout(bw):
        # [ci, 1152]: 3 DoubleRow groups (taps (0,g),(1,g)) then 3 singles
        # (taps (2,g)); within a group the two taps' [ci, co] blocks are
        # adjacent (matching the lhsT [K, 2, M] access pattern).
        m = np.zeros((P, 9 * P), np.float64)
        t = bw.transpose(2, 3, 1, 0)  # [ky, kx, ci, co]
        for g in range(3):
            m[:, 256 * g:256 * g + 128] = t[0, g]
            m[:, 256 * g + 128:256 * g + 256] = t[1, g]
            m[:, 768 + 128 * g:768 + 128 * (g + 1)] = t[2, g]
        return m

    w1m = wlayout(bw1)

    # conv2 channel permutation (both in and out sides)
    pidx = np.arange(P)
    chan = np.where(pidx < 64, 2 * pidx, 2 * (pidx - 64) + 1)
    bw2p = bw2[np.ix_(chan, chan)]
    w2m = wlayout(bw2p)

    wcb = np.concatenate([w1m, w2m], axis=1)
    wcb = np.ascontiguousarray(wcb).astype(ml_dtypes.float8_e4m3)

    # u-domain: conv_sign = 2*conv_u - c0, c0 = sum of signed weights
    inv1 = bn1_gamma.astype(f8) / np.sqrt(bn1_var.astype(f8) + EPS)
    c0_1 = bw1.sum(axis=(1, 2, 3))
    s1 = 2.0 * inv1
    b1 = bn1_beta.astype(f8) - bn1_mean.astype(f8) * inv1 - inv1 * c0_1

    inv2 = (bn2_gamma.astype(f8) / np.sqrt(bn2_var.astype(f8) + EPS))[chan]
    c0_2 = bw2.sum(axis=(1, 2, 3))[chan]
    s2 = 2.0 * inv2
    b2 = bn2_beta.astype(f8)[chan] - bn2_mean.astype(f8)[chan] * inv2 - inv2 * c0_2

    i = np.arange(64)
    m1e = move1_bias[2 * i]            # fo1 bias (conv1-out hi ch 64+i)
    mv0l = move0_bias[i]               # idle-lo bias
    cst = np.zeros((P, 16), np.float64)
    cst[:, 0] = s1
    cst[:, 1] = b1
    cst[64:128, 1] += m1e              # fold move1_even into conv1 hi bias
    cst[:, 2] = s2
    cst[:, 3] = b2
    cst[64:128, 3] += mv0l             # idle residual bias for conv2
    cst[64:128, 4] = 1.0 + m1e         # fo1 shifted clip bounds
    cst[64:128, 5] = -1.0 + m1e
    cst[64:128, 6] = -mv0l             # u2 threshold on idle partitions
    cst[0:64, 7] = move0_bias[64 + i] + move1_bias[2 * i + 1]
    dg2 = np.diag(1.0 / s2).astype(np.float16)
    return wcb, cst.astype(np.float32), dg2


def kernel(x, w1, w2, bn1_gamma, bn1_beta, bn1_mean, bn1_var,
           bn2_gamma, bn2_beta, bn2_mean, bn2_var, move0_bias, move1_bias,
           _trace=False):
    x = np.asarray(x, np.float32)
    args = [np.asarray(a, np.float32) for a in (
        w1, w2, bn1_gamma, bn1_beta, bn1_mean, bn1_var,
        bn2_gamma, bn2_beta, bn2_mean, bn2_var, move0_bias, move1_bias)]
    wcb, cst, dg2 = _host_prep(*args)

    if "nc" not in _CACHE:
        _CACHE["nc"] = _build()
    nc = _CACHE["nc"]

    in_maps = [
        {"xs": np.ascontiguousarray(x[IMGS_PER_CORE * c:IMGS_PER_CORE * (c + 1)]),
         "wcb": wcb, "cst": cst, "dg2": dg2}
        for c in range(NCORES)
    ]
    kw = {}
    if _trace:
        kw = dict(trace=True, trace_kwargs={"title": "basicblock"})
    res = bass_utils.run_bass_kernel_spmd(nc, in_maps, core_ids=list(range(NCORES)), **kw)
    out = np.concatenate([res.results[c]["out"] for c in range(NCORES)], axis=0)
    if _trace:
        _CACHE["last_results"] = res
    return out
